# revision 6
# baseline (speedup 1.0000x reference)
"""KNN space regularizer kernel for Trainium2 (8 NeuronCores, SPMD).

Data-parallel over batch B=8: one batch element per core.
Per core (N=4096 points, D=3), per 128-row tile:
  inner2 = PE fp32 matmul of lhsT=[2x0;2x1;2x2] vs rhs=[x0;x1;x2]  (= 2<xi,xj>)
  s = (-sq_j + -sq_i) + inner2   (DVE scalar_tensor_tensor, PSUM fused)
This reproduces XLA-Neuron's d2 = (sq_i+sq_j) - 2*inner bitwise (verified:
0/32768 rows differ from the on-device jax reference), so the top-k
selection matches the reference exactly; sqrt/clamp are monotone.
Top-k (k = argmax(k_vector)+1, computed on host like the torch .item())
selected per row with DVE max8 (+ match_replace round for k>8) and
max_index; preds rows gathered from DRAM via per-row indirect DMA;
mean written out.  sqrt/clamp of the reference are monotone so ordering
on -d2 matches ordering on the reference's distances.
"""

import os
import sys

import numpy as np

sys.path.insert(0, "/opt/trn_rl_repo")
sys.path.insert(0, "/opt/trn_rl_repo/concourse")

N = 4096
D = 3
P = 128
NT = N // P  # 32 row tiles
HALF = 2048  # psum half width
MM = 512  # matmul free chunk (one PSUM bank)
NCORES = 8

_CACHE = {}


def _build(k: int):
    import concourse.bass as bass
    import concourse.mybir as mybir
    import concourse.tile as tile
    from concourse import bacc

    f32 = mybir.dt.float32
    nc = bacc.Bacc(
        "TRN2",
        target_bir_lowering=False,
        debug=False,
        num_devices=NCORES,
    )

    augA = nc.dram_tensor("augA", [3, N], f32, kind="ExternalInput").ap()
    augB = nc.dram_tensor("augB", [3, N], f32, kind="ExternalInput").ap()
    negsqb = nc.dram_tensor("negsqb", [P, N], f32, kind="ExternalInput").ap()
    negsqc = nc.dram_tensor("negsqc", [P, NT], f32, kind="ExternalInput").ap()
    preds_d = nc.dram_tensor("preds", [N, D], f32, kind="ExternalInput").ap()
    out_d = nc.dram_tensor("out", [N, D], f32, kind="ExternalOutput").ap()

    STT_ENGINE = nc.vector  # Act engine has no scalar_tensor_tensor
    kk = min(k, 8)  # first-round take
    k2 = k - kk  # second-round take (k > 8)

    with tile.TileContext(nc) as tc:
        with (
            tc.tile_pool(name="const", bufs=1) as constp,
            tc.tile_pool(name="psum", bufs=2, space="PSUM") as psump,
            tc.tile_pool(name="sbig", bufs=2) as sp,
            tc.tile_pool(name="small", bufs=3) as smallp,
            tc.tile_pool(name="gath", bufs=2) as gp,
        ):
            A = constp.tile([3, N], f32)
            nc.sync.dma_start(A[:], augA[:])
            Bm = constp.tile([3, N], f32)
            nc.sync.dma_start(Bm[:], augB[:])
            nsb = constp.tile([P, N], f32)
            nc.sync.dma_start(nsb[:], negsqb[:])
            nsc = constp.tile([P, NT], f32)
            nc.sync.dma_start(nsc[:], negsqc[:])

            for t in range(NT):
                s_sb = sp.tile([P, N], f32, tag="s_sb")
                for h in range(2):
                    ps = psump.tile([P, HALF], f32, tag="ps")
                    for c in range(HALF // MM):
                        j0 = h * HALF + c * MM
                        nc.tensor.matmul(
                            ps[:, c * MM : (c + 1) * MM],
                            A[:, t * P : (t + 1) * P],
                            Bm[:, j0 : j0 + MM],
                            start=True,
                            stop=True,
                        )
                    STT_ENGINE.scalar_tensor_tensor(
                        out=s_sb[:, h * HALF : (h + 1) * HALF],
                        in0=nsb[:, h * HALF : (h + 1) * HALF],
                        scalar=nsc[:, t : t + 1],
                        in1=ps[:],
                        op0=mybir.AluOpType.add,
                        op1=mybir.AluOpType.add,
                    )

                val8 = smallp.tile([P, 8], f32, tag="val8")
                nc.vector.max(val8[:], s_sb[:])
                idx8 = smallp.tile([P, 8], mybir.dt.uint32, tag="idx8")
                nc.vector.max_index(idx8[:], val8[:], s_sb[:])

                g = gp.tile([P, k, D], f32, tag="g")
                for r in range(kk):
                    nc.gpsimd.indirect_dma_start(
                        out=g[:, r, :],
                        out_offset=None,
                        in_=preds_d[:],
                        in_offset=bass.IndirectOffsetOnAxis(
                            ap=idx8[:, r : r + 1], axis=0
                        ),
                    )

                if k2 > 0:
                    s_mr = sp.tile([P, N], f32, tag="s_mr")
                    nc.vector.match_replace(
                        out=s_mr[:],
                        in_to_replace=val8[:],
                        in_values=s_sb[:],
                        imm_value=-1e30,
                    )
                    val8b = smallp.tile([P, 8], f32, tag="val8b")
                    nc.vector.max(val8b[:], s_mr[:])
                    idx8b = smallp.tile([P, 8], mybir.dt.uint32, tag="idx8b")
                    nc.vector.max_index(idx8b[:], val8b[:], s_mr[:])
                    for r in range(k2):
                        nc.gpsimd.indirect_dma_start(
                            out=g[:, kk + r, :],
                            out_offset=None,
                            in_=preds_d[:],
                            in_offset=bass.IndirectOffsetOnAxis(
                                ap=idx8b[:, r : r + 1], axis=0
                            ),
                        )

                acc = smallp.tile([P, D], f32, tag="acc")
                nc.vector.tensor_add(acc[:], g[:, 0, :], g[:, 1, :])
                for r in range(2, k):
                    nc.vector.tensor_add(acc[:], acc[:], g[:, r, :])
                mo = smallp.tile([P, D], f32, tag="mo")
                nc.scalar.mul(mo[:], acc[:], 1.0 / k)
                nc.sync.dma_start(out_d[t * P : (t + 1) * P, :], mo[:])

    nc.compile()
    return nc


def kernel(x, preds, k_vector):
    x = np.ascontiguousarray(np.asarray(x), dtype=np.float32)
    preds = np.ascontiguousarray(np.asarray(preds), dtype=np.float32)
    k_vector = np.asarray(k_vector)
    k = int(np.argmax(k_vector)) + 1
    B = x.shape[0]
    assert x.shape == (B, N, D) and preds.shape == (B, N, D)

    if k not in _CACHE:
        if k == 1:
            # top-1 is just the self point (distance 0); mean == preds row
            _CACHE[k] = None
        else:
            _CACHE[k] = _build(k)
    if k == 1:
        return preds.copy()
    nc = _CACHE[k]

    sq = np.sum(x * x, axis=-1, dtype=np.float32)  # [B, N]
    negsq = -sq
    augA = np.stack(
        [2.0 * x[..., 0], 2.0 * x[..., 1], 2.0 * x[..., 2]], axis=1
    ).astype(np.float32)  # [B, 3, N]
    augB = np.stack([x[..., 0], x[..., 1], x[..., 2]], axis=1).astype(
        np.float32
    )  # [B, 3, N]
    # negsq broadcast [P, N] and column layout [P, NT] (col t, row p -> i = t*P+p)
    negsqb = np.broadcast_to(negsq[:, None, :], (B, P, N)).copy()
    negsqc = negsq.reshape(B, NT, P).transpose(0, 2, 1).copy()

    in_maps = [
        {
            "augA": np.ascontiguousarray(augA[b]),
            "augB": np.ascontiguousarray(augB[b]),
            "negsqb": np.ascontiguousarray(negsqb[b]),
            "negsqc": np.ascontiguousarray(negsqc[b]),
            "preds": np.ascontiguousarray(preds[b]),
        }
        for b in range(B)
    ]

    from concourse.bass_utils import run_bass_kernel_spmd

    res = run_bass_kernel_spmd(nc, in_maps, core_ids=list(range(NCORES)))
    out = np.stack([res.results[b]["out"] for b in range(B)], axis=0)
    return out.astype(np.float32)


if __name__ == "__main__":
    rng = np.random.default_rng(0)
    x = rng.standard_normal((8, N, D), dtype=np.float32)
    p = rng.standard_normal((8, N, D), dtype=np.float32)
    kv = rng.standard_normal((16,), dtype=np.float32)
    o = kernel(x, p, kv)
    print(o.shape, o.dtype)


# revision 7
# speedup vs baseline: 1.9202x; 1.9202x over previous
"""KNN space regularizer kernel for Trainium2 (8 NeuronCores, SPMD).

Data-parallel over batch B=8: one batch element per core.
Per core (N=4096 points, D=3), per 128-row tile:
  inner2 = PE fp32 matmul of lhsT=[2x0;2x1;2x2] vs rhs=[x0;x1;x2]  (= 2<xi,xj>)
  s = (-sq_j + -sq_i) + inner2   (DVE scalar_tensor_tensor, PSUM fused)
This reproduces XLA-Neuron's d2 = (sq_i+sq_j) - 2*inner bitwise (verified:
0/32768 rows differ from the on-device jax reference), so the top-k
selection matches the reference exactly; sqrt/clamp are monotone.
Top-k (k = argmax(k_vector)+1, computed on host like the torch .item())
selected per row with DVE max8 (+ match_replace round for k>8) and
max_index; preds rows gathered from DRAM via per-row indirect DMA;
mean written out.  sqrt/clamp of the reference are monotone so ordering
on -d2 matches ordering on the reference's distances.
"""

import os
import sys

import numpy as np

sys.path.insert(0, "/opt/trn_rl_repo")
sys.path.insert(0, "/opt/trn_rl_repo/concourse")

N = 4096
D = 3
P = 128
NT = N // P  # 32 row tiles
HALF = 2048  # psum half width
MM = 512  # matmul free chunk (one PSUM bank)
NCORES = 8

_CACHE = {}


def _build(k: int):
    import concourse.bass as bass
    import concourse.mybir as mybir
    import concourse.tile as tile
    from concourse import bacc

    f32 = mybir.dt.float32
    nc = bacc.Bacc(
        "TRN2",
        target_bir_lowering=False,
        debug=False,
        num_devices=NCORES,
    )

    augA = nc.dram_tensor("augA", [3, N], f32, kind="ExternalInput").ap()
    augB = nc.dram_tensor("augB", [3, N], f32, kind="ExternalInput").ap()
    negsqr = nc.dram_tensor("negsqr", [1, N], f32, kind="ExternalInput").ap()
    negsqc = nc.dram_tensor("negsqc", [P, NT], f32, kind="ExternalInput").ap()
    preds_d = nc.dram_tensor("preds", [N, D], f32, kind="ExternalInput").ap()
    out_d = nc.dram_tensor("out", [N, D], f32, kind="ExternalOutput").ap()

    STT_ENGINE = nc.vector  # Act engine has no scalar_tensor_tensor
    kk = min(k, 8)  # first-round take
    k2 = k - kk  # second-round take (k > 8)

    with tile.TileContext(nc) as tc:
        with (
            tc.tile_pool(name="const", bufs=1) as constp,
            tc.tile_pool(name="psum", bufs=2, space="PSUM") as psump,
            tc.tile_pool(name="sbig", bufs=2) as sp,
            tc.tile_pool(name="small", bufs=3) as smallp,
            tc.tile_pool(name="gath", bufs=2) as gp,
        ):
            A = constp.tile([3, N], f32)
            nc.sync.dma_start(A[:], augA[:])
            Bm = constp.tile([3, N], f32)
            nc.sync.dma_start(Bm[:], augB[:])
            nsr = constp.tile([1, N], f32)
            nc.sync.dma_start(nsr[:], negsqr[:])
            ones = constp.tile([1, P], f32)
            nc.gpsimd.memset(ones[:], 1.0)
            # broadcast -sq to all 128 partitions via K=1 ones-matmul
            # (1.0 * v is exact in fp32, so nsb rows are bitwise -sq)
            nsb = constp.tile([P, N], f32)
            for h in range(2):
                ps = psump.tile([P, HALF], f32, tag="ps")
                for c in range(HALF // MM):
                    j0 = h * HALF + c * MM
                    nc.tensor.matmul(
                        ps[:, c * MM : (c + 1) * MM],
                        ones[:],
                        nsr[:, j0 : j0 + MM],
                        start=True,
                        stop=True,
                    )
                nc.scalar.copy(nsb[:, h * HALF : (h + 1) * HALF], ps[:])
            nsc = constp.tile([P, NT], f32)
            nc.sync.dma_start(nsc[:], negsqc[:])

            for t in range(NT):
                s_sb = sp.tile([P, N], f32, tag="s_sb")
                for h in range(2):
                    ps = psump.tile([P, HALF], f32, tag="ps")
                    for c in range(HALF // MM):
                        j0 = h * HALF + c * MM
                        nc.tensor.matmul(
                            ps[:, c * MM : (c + 1) * MM],
                            A[:, t * P : (t + 1) * P],
                            Bm[:, j0 : j0 + MM],
                            start=True,
                            stop=True,
                        )
                    STT_ENGINE.scalar_tensor_tensor(
                        out=s_sb[:, h * HALF : (h + 1) * HALF],
                        in0=nsb[:, h * HALF : (h + 1) * HALF],
                        scalar=nsc[:, t : t + 1],
                        in1=ps[:],
                        op0=mybir.AluOpType.add,
                        op1=mybir.AluOpType.add,
                    )

                val8 = smallp.tile([P, 8], f32, tag="val8")
                nc.vector.max(val8[:], s_sb[:])
                idx8 = smallp.tile([P, 8], mybir.dt.uint32, tag="idx8")
                nc.vector.max_index(idx8[:], val8[:], s_sb[:])

                g = gp.tile([P, k, D], f32, tag="g")
                for r in range(kk):
                    nc.gpsimd.indirect_dma_start(
                        out=g[:, r, :],
                        out_offset=None,
                        in_=preds_d[:],
                        in_offset=bass.IndirectOffsetOnAxis(
                            ap=idx8[:, r : r + 1], axis=0
                        ),
                    )

                if k2 > 0:
                    s_mr = sp.tile([P, N], f32, tag="s_mr")
                    nc.vector.match_replace(
                        out=s_mr[:],
                        in_to_replace=val8[:],
                        in_values=s_sb[:],
                        imm_value=-1e30,
                    )
                    val8b = smallp.tile([P, 8], f32, tag="val8b")
                    nc.vector.max(val8b[:], s_mr[:])
                    idx8b = smallp.tile([P, 8], mybir.dt.uint32, tag="idx8b")
                    nc.vector.max_index(idx8b[:], val8b[:], s_mr[:])
                    for r in range(k2):
                        nc.gpsimd.indirect_dma_start(
                            out=g[:, kk + r, :],
                            out_offset=None,
                            in_=preds_d[:],
                            in_offset=bass.IndirectOffsetOnAxis(
                                ap=idx8b[:, r : r + 1], axis=0
                            ),
                        )

                acc = smallp.tile([P, D], f32, tag="acc")
                nc.vector.tensor_add(acc[:], g[:, 0, :], g[:, 1, :])
                for r in range(2, k):
                    nc.vector.tensor_add(acc[:], acc[:], g[:, r, :])
                mo = smallp.tile([P, D], f32, tag="mo")
                nc.scalar.mul(mo[:], acc[:], 1.0 / k)
                nc.sync.dma_start(out_d[t * P : (t + 1) * P, :], mo[:])

    nc.compile()
    return nc


def kernel(x, preds, k_vector):
    x = np.ascontiguousarray(np.asarray(x), dtype=np.float32)
    preds = np.ascontiguousarray(np.asarray(preds), dtype=np.float32)
    k_vector = np.asarray(k_vector)
    k = int(np.argmax(k_vector)) + 1
    B = x.shape[0]
    assert x.shape == (B, N, D) and preds.shape == (B, N, D)

    if k not in _CACHE:
        if k == 1:
            # top-1 is just the self point (distance 0); mean == preds row
            _CACHE[k] = None
        else:
            _CACHE[k] = _build(k)
    if k == 1:
        return preds.copy()
    nc = _CACHE[k]

    sq = np.sum(x * x, axis=-1, dtype=np.float32)  # [B, N]
    negsq = -sq
    augA = np.stack(
        [2.0 * x[..., 0], 2.0 * x[..., 1], 2.0 * x[..., 2]], axis=1
    ).astype(np.float32)  # [B, 3, N]
    augB = np.stack([x[..., 0], x[..., 1], x[..., 2]], axis=1).astype(
        np.float32
    )  # [B, 3, N]
    # negsq row [1, N] (broadcast on-chip) and column layout [P, NT]
    negsqc = negsq.reshape(B, NT, P).transpose(0, 2, 1).copy()

    in_maps = [
        {
            "augA": np.ascontiguousarray(augA[b]),
            "augB": np.ascontiguousarray(augB[b]),
            "negsqr": np.ascontiguousarray(negsq[b][None, :]),
            "negsqc": np.ascontiguousarray(negsqc[b]),
            "preds": np.ascontiguousarray(preds[b]),
        }
        for b in range(B)
    ]

    from concourse.bass_utils import run_bass_kernel_spmd

    res = run_bass_kernel_spmd(nc, in_maps, core_ids=list(range(NCORES)))
    out = np.stack([res.results[b]["out"] for b in range(B)], axis=0)
    return out.astype(np.float32)


if __name__ == "__main__":
    rng = np.random.default_rng(0)
    x = rng.standard_normal((8, N, D), dtype=np.float32)
    p = rng.standard_normal((8, N, D), dtype=np.float32)
    kv = rng.standard_normal((16,), dtype=np.float32)
    o = kernel(x, p, kv)
    print(o.shape, o.dtype)


# revision 8
# speedup vs baseline: 5.1830x; 2.6992x over previous
"""KNN space regularizer kernel for Trainium2 (8 NeuronCores, SPMD).

Data-parallel over batch B=8: one batch element per core.
Per core (N=4096 points, D=3), per 128-row tile:
  inner2 = PE fp32 matmul of lhsT=[2x0;2x1;2x2] vs rhs=[x0;x1;x2]  (= 2<xi,xj>)
  s = (-sq_j + -sq_i) + inner2   (DVE scalar_tensor_tensor, PSUM fused)
This reproduces XLA-Neuron's d2 = (sq_i+sq_j) - 2*inner bitwise (verified:
0/32768 rows differ from the on-device jax reference), so the top-k
selection matches the reference exactly; sqrt/clamp are monotone.
Top-k (k = argmax(k_vector)+1, computed on host like the torch .item())
selected per row with DVE max8 (+ match_replace round for k>8) and
max_index; preds rows gathered from DRAM via per-row indirect DMA;
mean written out.  sqrt/clamp of the reference are monotone so ordering
on -d2 matches ordering on the reference's distances.
"""

import os
import sys

import numpy as np

sys.path.insert(0, "/opt/trn_rl_repo")
sys.path.insert(0, "/opt/trn_rl_repo/concourse")

N = 4096
D = 3
P = 128
NT = N // P  # 32 row tiles
HALF = 2048  # psum half width
MM = 512  # matmul free chunk (one PSUM bank)
NCORES = 8

_CACHE = {}


def _build(k: int):
    import concourse.bass as bass
    import concourse.mybir as mybir
    import concourse.tile as tile
    from concourse import bacc

    f32 = mybir.dt.float32
    nc = bacc.Bacc(
        "TRN2",
        target_bir_lowering=False,
        debug=False,
        num_devices=NCORES,
    )

    augA = nc.dram_tensor("augA", [3, N], f32, kind="ExternalInput").ap()
    augB = nc.dram_tensor("augB", [3, N], f32, kind="ExternalInput").ap()
    negsqr = nc.dram_tensor("negsqr", [1, N], f32, kind="ExternalInput").ap()
    negsqc = nc.dram_tensor("negsqc", [P, NT], f32, kind="ExternalInput").ap()
    preds_d = nc.dram_tensor("preds", [N, D], f32, kind="ExternalInput").ap()
    out_d = nc.dram_tensor("out", [N, D], f32, kind="ExternalOutput").ap()

    STT_ENGINE = nc.vector  # Act engine has no scalar_tensor_tensor
    kk = min(k, 8)  # first-round take
    k2 = k - kk  # second-round take (k > 8)

    with tile.TileContext(nc) as tc:
        with (
            tc.tile_pool(name="const", bufs=1) as constp,
            tc.tile_pool(name="psum", bufs=2, space="PSUM") as psump,
            tc.tile_pool(name="sbig", bufs=2) as sp,
            tc.tile_pool(name="small", bufs=3) as smallp,
            tc.tile_pool(name="gath", bufs=2) as gp,
        ):
            A = constp.tile([3, N], f32)
            nc.sync.dma_start(A[:], augA[:])
            Bm = constp.tile([3, N], f32)
            nc.sync.dma_start(Bm[:], augB[:])
            nsr = constp.tile([1, N], f32)
            nc.sync.dma_start(nsr[:], negsqr[:])
            ones = constp.tile([1, P], f32)
            nc.gpsimd.memset(ones[:], 1.0)
            # broadcast -sq to all 128 partitions via K=1 ones-matmul
            # (1.0 * v is exact in fp32, so nsb rows are bitwise -sq)
            nsb = constp.tile([P, N], f32)
            for h in range(2):
                ps = psump.tile([P, HALF], f32, tag="ps")
                for c in range(HALF // MM):
                    j0 = h * HALF + c * MM
                    nc.tensor.matmul(
                        ps[:, c * MM : (c + 1) * MM],
                        ones[:],
                        nsr[:, j0 : j0 + MM],
                        start=True,
                        stop=True,
                    )
                nc.scalar.copy(nsb[:, h * HALF : (h + 1) * HALF], ps[:])
            nsc = constp.tile([P, NT], f32)
            nc.sync.dma_start(nsc[:], negsqc[:])

            for t in range(NT):
                s_sb = sp.tile([P, N], f32, tag="s_sb")
                for h in range(2):
                    ps = psump.tile([P, HALF], f32, tag="ps")
                    for c in range(HALF // MM):
                        j0 = h * HALF + c * MM
                        nc.tensor.matmul(
                            ps[:, c * MM : (c + 1) * MM],
                            A[:, t * P : (t + 1) * P],
                            Bm[:, j0 : j0 + MM],
                            start=True,
                            stop=True,
                        )
                    STT_ENGINE.scalar_tensor_tensor(
                        out=s_sb[:, h * HALF : (h + 1) * HALF],
                        in0=nsb[:, h * HALF : (h + 1) * HALF],
                        scalar=nsc[:, t : t + 1],
                        in1=ps[:],
                        op0=mybir.AluOpType.add,
                        op1=mybir.AluOpType.add,
                    )

                val8 = smallp.tile([P, 8], f32, tag="val8")
                nc.vector.max(val8[:], s_sb[:])
                idx8 = smallp.tile([P, 8], mybir.dt.uint32, tag="idx8")
                nc.vector.max_index(idx8[:], val8[:], s_sb[:])

                g = gp.tile([P, k, D], f32, tag="g")
                for r in range(kk):
                    nc.gpsimd.indirect_dma_start(
                        out=g[:, r, :],
                        out_offset=None,
                        in_=preds_d[:],
                        in_offset=bass.IndirectOffsetOnAxis(
                            ap=idx8[:, r : r + 1], axis=0
                        ),
                    )

                if k2 > 0:
                    s_mr = sp.tile([P, N], f32, tag="s_mr")
                    nc.vector.match_replace(
                        out=s_mr[:],
                        in_to_replace=val8[:],
                        in_values=s_sb[:],
                        imm_value=-1e30,
                    )
                    val8b = smallp.tile([P, 8], f32, tag="val8b")
                    nc.vector.max(val8b[:], s_mr[:])
                    idx8b = smallp.tile([P, 8], mybir.dt.uint32, tag="idx8b")
                    nc.vector.max_index(idx8b[:], val8b[:], s_mr[:])
                    for r in range(k2):
                        nc.gpsimd.indirect_dma_start(
                            out=g[:, kk + r, :],
                            out_offset=None,
                            in_=preds_d[:],
                            in_offset=bass.IndirectOffsetOnAxis(
                                ap=idx8b[:, r : r + 1], axis=0
                            ),
                        )

                acc = smallp.tile([P, D], f32, tag="acc")
                nc.vector.tensor_add(acc[:], g[:, 0, :], g[:, 1, :])
                for r in range(2, k):
                    nc.vector.tensor_add(acc[:], acc[:], g[:, r, :])
                mo = smallp.tile([P, D], f32, tag="mo")
                nc.scalar.mul(mo[:], acc[:], 1.0 / k)
                nc.sync.dma_start(out_d[t * P : (t + 1) * P, :], mo[:])

    nc.compile()
    return nc


def _make_runner(nc):
    """Build the shard_map-jitted executor ONCE per compiled module.

    run_bass_kernel_spmd rebuilds jax.jit(shard_map(...)) on every call
    (~250ms of dispatch/lowering overhead); caching it amortizes that.
    Mirrors concourse.bass2jax.run_bass_via_pjrt.
    """
    import jax
    from jax.experimental.shard_map import shard_map
    from jax.sharding import Mesh, PartitionSpec

    import concourse.mybir as mybir
    from concourse import bass2jax

    bass2jax.install_neuronx_cc_hook()
    assert nc.dbg_addr is None  # built with debug=False
    partition_name = (
        nc.partition_id_tensor.name if nc.partition_id_tensor else None
    )
    in_names, out_names, out_avals = [], [], []
    for alloc in nc.m.functions[0].allocations:
        if not isinstance(alloc, mybir.MemoryLocationSet):
            continue
        name = alloc.memorylocations[0].name
        if alloc.kind == "ExternalInput":
            if name != partition_name:
                in_names.append(name)
        elif alloc.kind == "ExternalOutput":
            out_names.append(name)
            shape = tuple(alloc.tensor_shape)
            dtype = mybir.dt.np(alloc.dtype)
            out_avals.append(jax.core.ShapedArray(shape, dtype))
    n_params = len(in_names)
    n_outs = len(out_avals)
    in_names = in_names + out_names + ([partition_name] if partition_name else [])
    donate = tuple(range(n_params, n_params + n_outs))

    def _body(*args):
        operands = list(args)
        if partition_name is not None:
            operands.append(bass2jax.partition_id_tensor())
        outs = bass2jax._bass_exec_p.bind(
            *operands,
            out_avals=tuple(out_avals),
            in_names=tuple(in_names),
            out_names=tuple(out_names),
            lowering_input_output_aliases=(),
            sim_require_finite=True,
            sim_require_nnan=True,
            nc=nc,
        )
        return tuple(outs)

    devices = jax.devices()[:NCORES]
    mesh = Mesh(np.asarray(devices), ("core",))
    in_specs = (PartitionSpec("core"),) * (n_params + n_outs)
    out_specs = (PartitionSpec("core"),) * n_outs
    sharded = jax.jit(
        shard_map(
            _body, mesh=mesh, in_specs=in_specs, out_specs=out_specs,
            check_rep=False,
        ),
        donate_argnums=donate,
        keep_unused=True,
    )
    param_names = in_names[:n_params]

    def run(in_maps):
        concat_in = [
            np.concatenate([np.asarray(m[name]) for m in in_maps], axis=0)
            for name in param_names
        ]
        concat_zeros = [
            np.zeros((NCORES * a.shape[0], *a.shape[1:]), a.dtype)
            for a in out_avals
        ]
        out_arrs = sharded(*concat_in, *concat_zeros)
        return [
            {
                name: np.asarray(out_arrs[i]).reshape(
                    NCORES, *out_avals[i].shape
                )[c]
                for i, name in enumerate(out_names)
            }
            for c in range(NCORES)
        ]

    return run


def kernel(x, preds, k_vector):
    x = np.ascontiguousarray(np.asarray(x), dtype=np.float32)
    preds = np.ascontiguousarray(np.asarray(preds), dtype=np.float32)
    k_vector = np.asarray(k_vector)
    k = int(np.argmax(k_vector)) + 1
    B = x.shape[0]
    assert x.shape == (B, N, D) and preds.shape == (B, N, D)

    if k not in _CACHE:
        if k == 1:
            # top-1 is just the self point (distance 0); mean == preds row
            _CACHE[k] = None
        else:
            nc = _build(k)
            try:
                runner = _make_runner(nc)
            except Exception:
                runner = None
            _CACHE[k] = (nc, runner)
    if k == 1:
        return preds.copy()
    nc, runner = _CACHE[k]

    sq = np.sum(x * x, axis=-1, dtype=np.float32)  # [B, N]
    negsq = -sq
    augA = np.stack(
        [2.0 * x[..., 0], 2.0 * x[..., 1], 2.0 * x[..., 2]], axis=1
    ).astype(np.float32)  # [B, 3, N]
    augB = np.stack([x[..., 0], x[..., 1], x[..., 2]], axis=1).astype(
        np.float32
    )  # [B, 3, N]
    # negsq row [1, N] (broadcast on-chip) and column layout [P, NT]
    negsqc = negsq.reshape(B, NT, P).transpose(0, 2, 1).copy()

    in_maps = [
        {
            "augA": np.ascontiguousarray(augA[b]),
            "augB": np.ascontiguousarray(augB[b]),
            "negsqr": np.ascontiguousarray(negsq[b][None, :]),
            "negsqc": np.ascontiguousarray(negsqc[b]),
            "preds": np.ascontiguousarray(preds[b]),
        }
        for b in range(B)
    ]

    results = None
    if runner is not None:
        try:
            results = runner(in_maps)
        except Exception:
            results = None
    if results is None:
        from concourse.bass_utils import run_bass_kernel_spmd

        results = run_bass_kernel_spmd(
            nc, in_maps, core_ids=list(range(NCORES))
        ).results
    out = np.stack([results[b]["out"] for b in range(B)], axis=0)
    return out.astype(np.float32)


if __name__ == "__main__":
    rng = np.random.default_rng(0)
    x = rng.standard_normal((8, N, D), dtype=np.float32)
    p = rng.standard_normal((8, N, D), dtype=np.float32)
    kv = rng.standard_normal((16,), dtype=np.float32)
    o = kernel(x, p, kv)
    print(o.shape, o.dtype)
